# revision 21
# baseline (speedup 1.0000x reference)
"""Sparse (sigmoid) attention block on 8 TRN2 NeuronCores.

Sharding: core c = (batch b=c//2, head-half hh=c%2).  Each core computes
QKV projection + RoPE + causal sigmoid-attention for its 6 heads over the
full 2048-row sequence of its batch; a per-row-block AllGather inside the
pair exchanges attention outputs so each core sees the full hidden dim,
then LayerNorm + silu(U) gating run locally and the output projection is
column-split across the pair (single collective per block, fired at the
earliest possible moment so it hides under the next attention block).

Layouts: Q^T/K^T are produced DIRECTLY in transposed [head-pair-dim, seq]
layout by making the projection weights PE-stationary; RoPE's rotate-half
becomes a host-built 128x128 permutation matmul (P @ Q^T) so no PE
transposes are needed anywhere.  Causal structure is exploited at 128-row
granularity inside each 512-query block: diagonal key-chunks trim their
scores / sigmoid / A@V work to the unmasked column range.  The LN mean is
broadcast via a tiny PE matmul with a constant -1/H stationary vector;
the 1/std scale is applied AFTER the output projection (it commutes with
the contraction) so the sqrt chain stays off the critical path.  All
heavy compute in bf16 with f32 PSUM accumulation.
"""

import numpy as np
import ml_dtypes

import concourse.bass as bass
import concourse.bacc as bacc
import concourse.mybir as mybir
import concourse.tile as tile
from concourse import bass_utils

BF16 = mybir.dt.bfloat16
F32 = mybir.dt.float32
AF = mybir.ActivationFunctionType

S = 2048          # sequence length
HID = 768         # hidden
D = 64            # head dim
NH = 6            # heads per core
NPAIR = 3         # head pairs per core
NRB = 4           # row blocks of 512
RB = 512
LN_EPS = 1e-8
N_CORES = 8


def _rope_tables():
    inv_freq = 1.0 / (10000.0 ** (np.arange(0, D, 2, dtype=np.float64) / D))
    t = np.arange(S, dtype=np.float64)
    freqs = np.outer(t, inv_freq)                      # [S, 32]
    emb = np.concatenate([freqs, freqs], axis=-1)      # [S, 64]
    return np.cos(emb).astype(np.float32), np.sin(emb).astype(np.float32)


def build_nc(ndev, pairs):
    """Emit the per-core Bass/Tile graph (identical for every core)."""
    nc = bacc.Bacc("TRN2", target_bir_lowering=False, debug=False,
                   num_devices=ndev)

    def din(name, shape, dt):
        return nc.dram_tensor(name, shape, dt, kind="ExternalInput").ap()

    xT = din("xT", [HID, S], BF16)
    wq = din("wq", [HID, 384], BF16)
    wk = din("wk", [HID, 384], BF16)
    wv = din("wv", [HID, 384], BF16)
    wu = din("wu", [HID, HID], BF16)                   # full U cols
    wo = din("wo", [HID, 384], BF16)                   # full rows, own cols
    cosT = din("cosT", [128, S], BF16)
    sinT = din("sinT", [128, S], BF16)                 # sign-folded sin^T
    pmat = din("pmat", [128, 128], BF16)               # rotate-half perm
    maskb = din("maskb", [128, 128], BF16)             # j>=i upper-tri
    ones_k = din("ones_k", [128, 1], BF16)
    residT = din("residT", [384, S], F32)              # x^T half + b_out
    out = nc.dram_tensor("out", [384, S], F32, kind="ExternalOutput").ap()

    xT_r = xT.rearrange("(k p) s -> p k s", p=128)     # [128, 6, S]
    wq_r = wq.rearrange("(k p) c -> p k c", p=128)     # [128, 6, 384]
    wk_r = wk.rearrange("(k p) c -> p k c", p=128)
    wv_r = wv.rearrange("(k p) c -> p k c", p=128)
    wu_r = wu.rearrange("(k p) c -> p k c", p=128)     # [128, 6, 768]
    wo_r = wo.rearrange("(k p) c -> p k c", p=128)     # [128, 6, 384]
    residT_r = residT.rearrange("(c p) s -> p c s", p=128)  # [128, 3, S]
    out_r = out.rearrange("(c p) s -> p c s", p=128)

    with tile.TileContext(nc) as tc:
        _emit(nc, tc, pairs, xT_r, wq_r, wk_r, wv_r, wu_r, wo_r,
              cosT, sinT, pmat, maskb, ones_k, residT_r, out_r)
    nc.compile()
    return nc


def _emit(nc, tc, pairs, xT_r, wq_r, wk_r, wv_r, wu_r, wo_r,
          cosT, sinT, pmat, maskb, ones_k, residT_r, out_r):
    from contextlib import ExitStack
    es = ExitStack()
    with es:
        # ---- resident SBUF tensors -----------------------------------
        res = es.enter_context(tc.tile_pool(name="resident", bufs=1))
        xT_sb = res.tile([128, 6, S], BF16, tag="xT")
        wq_sb = res.tile([128, 6, 384], BF16, tag="wq")
        wk_sb = res.tile([128, 6, 384], BF16, tag="wk")
        wv_sb = res.tile([128, 6, 384], BF16, tag="wv")
        wu_sb = res.tile([128, 6, HID], BF16, tag="wu")
        wo_sb = res.tile([128, 6, 384], BF16, tag="wo")
        cosT_sb = res.tile([128, S], BF16, tag="cosT")
        sinT_sb = res.tile([128, S], BF16, tag="sinT")
        pmat_sb = res.tile([128, 128], BF16, tag="pmat")
        maskb_sb = res.tile([128, 128], BF16, tag="maskb")
        ones_k_sb = res.tile([128, 1], BF16, tag="onesk")
        warm_sb = res.tile([128, 128], BF16, tag="warm")
        kt_sb = [res.tile([128, NPAIR, RB], BF16, tag=f"kt{i}", name=f"kt{i}")
                 for i in range(NRB)]                  # K^T slabs (roped)
        v_sb = [res.tile([128, 4, NH * D], BF16, tag=f"v{i}", name=f"v{i}")
                for i in range(NRB)]                   # V row-major slabs
        ut_sb = res.tile([128, 6, S], BF16, tag="ut")  # silu(U)^T full

        # critical-path loads first: wq + first x block start the PE
        nc.gpsimd.memset(warm_sb[:], 0.0)
        for k in range(6):
            nc.sync.dma_start(out=wq_sb[:, k, :], in_=wq_r[:, k, :])
        for k in range(6):
            nc.sync.dma_start(out=xT_sb[:, k, 0:RB], in_=xT_r[:, k, 0:RB])
        nc.sync.dma_start(out=pmat_sb[:], in_=pmat[:])
        nc.sync.dma_start(out=cosT_sb[:], in_=cosT[:])
        nc.sync.dma_start(out=sinT_sb[:], in_=sinT[:])
        for k in range(6):
            nc.sync.dma_start(out=wk_sb[:, k, :], in_=wk_r[:, k, :])
            nc.sync.dma_start(out=wv_sb[:, k, :], in_=wv_r[:, k, :])
            nc.sync.dma_start(out=wu_sb[:, k, :], in_=wu_r[:, k, :])
        nc.sync.dma_start(out=maskb_sb[:], in_=maskb[:])
        nc.sync.dma_start(out=ones_k_sb[:], in_=ones_k[:])
        for rb in range(1, NRB):
            for k in range(6):
                nc.sync.dma_start(out=xT_sb[:, k, rb * RB:(rb + 1) * RB],
                                  in_=xT_r[:, k, rb * RB:(rb + 1) * RB])
        for k in range(6):
            nc.sync.dma_start(out=wo_sb[:, k, :], in_=wo_r[:, k, :])

        # ---- pools ---------------------------------------------------
        dram = es.enter_context(tc.tile_pool(name="ccdram", bufs=4,
                                             space="DRAM"))
        projp = es.enter_context(tc.tile_pool(name="projp", bufs=2,
                                              space="PSUM"))      # 2 banks
        scp = es.enter_context(tc.tile_pool(name="scp", bufs=2,
                                            space="PSUM"))        # 4 banks
        avp = es.enter_context(tc.tile_pool(name="avp", bufs=1,
                                            space="PSUM"))        # 1 bank
        opo = es.enter_context(tc.tile_pool(name="opo", bufs=1,
                                            space="PSUM"))        # 1 bank
        psb = es.enter_context(tc.tile_pool(name="psb", bufs=2))
        atp = es.enter_context(tc.tile_pool(name="atp", bufs=8))
        qtp = es.enter_context(tc.tile_pool(name="qtp", bufs=2))
        aop = es.enter_context(tc.tile_pool(name="aop", bufs=2))
        esb = es.enter_context(tc.tile_pool(name="esb", bufs=2))
        ssb = es.enter_context(tc.tile_pool(name="ssb", bufs=1))

        eps_t = ssb.tile([1, 1], F32, tag="eps")
        nc.gpsimd.memset(eps_t[:], LN_EPS)
        negk = ssb.tile([1, 128], BF16, tag="negk")    # bcast lhsT: -1/HID
        nc.gpsimd.memset(negk[:], -1.0 / HID)

        # PE warm-up: ramp the clock while the first DMAs land
        wp = projp.tile([128, RB], F32, tag="pq", name="warm")
        for i in range(16):
            nc.tensor.matmul(wp[:, 0:128], warm_sb[:], warm_sb[:],
                             start=True, stop=True)

        # ---------------- per-phase emitters --------------------------
        def proj_qk(rb):
            """Q^T,K^T directly via W-stationary matmuls + perm-RoPE."""
            r0, r1 = rb * RB, (rb + 1) * RB
            qt = qtp.tile([128, NPAIR, RB], BF16, tag="qt", name=f"qt{rb}")
            units = [(wq_sb, qt, p) for p in range(NPAIR)]
            units += [(wk_sb, kt_sb[rb], p) for p in range(NPAIR)]
            # interleave: Q(p)/K(p) pq accumulation covers the previous
            # unit's PSUM->SBUF copy so the perm matmul never stalls
            staged = []
            for w_sb, dst, p in units:
                pq = projp.tile([128, RB], F32, tag="pq", name=f"pq{p}")
                for k in range(6):
                    nc.tensor.matmul(pq[:], w_sb[:, k, p * 128:(p + 1) * 128],
                                     xT_sb[:, k, r0:r1],
                                     start=(k == 0), stop=(k == 5))
                qsb = psb.tile([128, RB], BF16, tag="qsb")
                nc.scalar.copy(qsb[:], pq[:])
                staged.append((qsb, dst, p))
                if len(staged) == 2:
                    _finish_qk(staged.pop(0), r0, r1)
            while staged:
                _finish_qk(staged.pop(0), r0, r1)
            return qt

        def _finish_qk(st, r0, r1):
            qsb, dst, p = st
            pperm = projp.tile([128, RB], F32, tag="pq", name="pperm")
            nc.tensor.matmul(pperm[:], pmat_sb[:], qsb[:],
                             start=True, stop=True)
            t1 = psb.tile([128, RB], BF16, tag="t1")
            t2 = psb.tile([128, RB], BF16, tag="t2")
            nc.vector.tensor_mul(t1[:], qsb[:], cosT_sb[:, r0:r1])
            nc.vector.tensor_mul(t2[:], pperm[:], sinT_sb[:, r0:r1])
            nc.vector.tensor_add(dst[:, p, :], t1[:], t2[:])

        def proj_vu(rb):
            r0 = rb * RB
            for rt4 in range(4):
                c0 = r0 + rt4 * 128
                pv = projp.tile([128, RB], F32, tag="pq", name="pv")
                for k in range(6):
                    nc.tensor.matmul(pv[:, 0:384], xT_sb[:, k, c0:c0 + 128],
                                     wv_sb[:, k, :],
                                     start=(k == 0), stop=(k == 5))
                nc.scalar.copy(v_sb[rb][:, rt4, :], pv[:, 0:384])
            for ct in range(6):
                pu = projp.tile([128, RB], F32, tag="pq", name="pu")
                for k in range(6):
                    nc.tensor.matmul(pu[:], wu_sb[:, k, ct * 128:(ct + 1) * 128],
                                     xT_sb[:, k, r0:r0 + RB],
                                     start=(k == 0), stop=(k == 5))
                usig = psb.tile([128, RB], BF16, tag="usig")
                nc.scalar.activation(usig[:], pu[:], AF.Sigmoid)
                nc.vector.tensor_mul(ut_sb[:, ct, r0:r0 + RB], usig[:], pu[:])

        def attn(qb, qt):
            """Causal sigmoid attention for query block qb, all pairs.
            Stages each pair's output into the AllGather input as soon as
            it is ready and fires the collective at the end."""
            nkc = 4 * qb + 4
            ag_in = dram.tile([NPAIR, 128, RB], BF16, tag="agin")
            ag_out = dram.tile([2, NPAIR, 128, RB], BF16, tag="agout")
            ao = aop.tile([128, NPAIR, RB], BF16, tag="ao", name=f"ao{qb}")
            for p in range(NPAIR):
                av = avp.tile([128, RB], F32, tag="av")
                ats = {}

                def _av(kc):
                    t = kc - 4 * qb
                    w0 = max(t, 0) * 128
                    at = ats.pop(kc)
                    for h01 in range(2):
                        b0 = 64 * h01
                        nc.tensor.matmul(
                            av[b0:b0 + 64, w0:RB],
                            v_sb[kc // 4][:, kc % 4,
                                          (2 * p + h01) * D:(2 * p + h01 + 1) * D],
                            at[:, h01, w0:RB],
                            start=(kc == 0), stop=(kc == nkc - 1),
                            skip_group_check=True)

                for kc in range(nkc):
                    t = kc - 4 * qb          # >=0: diagonal-region chunk
                    w0 = max(t, 0) * 128
                    sc = scp.tile([128, 2, RB], F32, tag="sc")
                    at = atp.tile([128, 2, RB], BF16, tag="at")
                    kslc = kt_sb[kc // 4]
                    c0 = (kc % 4) * 128
                    for h01 in range(2):
                        b0 = 64 * h01
                        nc.tensor.matmul(
                            sc[:, h01, w0:RB],
                            kslc[b0:b0 + 64, p, c0:c0 + 128],
                            qt[b0:b0 + 64, p, w0:RB],
                            start=True, stop=True)
                    nc.scalar.activation(at[:, :, w0:RB], sc[:, :, w0:RB],
                                         AF.Sigmoid, scale=0.125)
                    if t >= 0:
                        for h01 in range(2):
                            nc.vector.tensor_mul(at[:, h01, w0:w0 + 128],
                                                 at[:, h01, w0:w0 + 128],
                                                 maskb_sb[:])
                    ats[kc] = at
                    if kc >= 7:              # bound live `at` tiles
                        _av(kc - 7)
                for kc in sorted(ats):
                    _av(kc)
                nc.vector.tensor_copy(ao[:, p, :], av[:])
                nc.gpsimd.dma_start(out=ag_in[p, :, :], in_=ao[:, p, :])
            nc.gpsimd.collective_compute(
                "AllGather", mybir.AluOpType.bypass, replica_groups=pairs,
                ins=[ag_in.opt()], outs=[ag_out.opt()])
            return ag_out

        def epilogue(rb, ag_out):
            """Full-hidden LN + gate locally; out-proj own column half."""
            r0, r1 = rb * RB, (rb + 1) * RB
            aot = esb.tile([128, 2, NPAIR, RB], BF16, tag="aot")
            nc.sync.dma_start(out=aot[:],
                              in_=ag_out.rearrange("r p i j -> i r p j"))
            aotf = aot.rearrange("i r p j -> i (r p) j")   # [128, 6, RB]
            ssum = projp.tile([1, RB], F32, tag="pq", name=f"ssum{rb}")
            qsum = projp.tile([1, RB], F32, tag="pq", name=f"qsum{rb}")
            for ct in range(6):
                sq = psb.tile([128, RB], BF16, tag="sq")
                nc.vector.tensor_mul(sq[:], aotf[:, ct, :], aotf[:, ct, :])
                nc.tensor.matmul(ssum[:], ones_k_sb[:], aotf[:, ct, :],
                                 start=(ct == 0), stop=(ct == 5))
                nc.tensor.matmul(qsum[:], ones_k_sb[:], sq[:],
                                 start=(ct == 0), stop=(ct == 5))
            stats = ssb.tile([1, 2, RB], F32, tag="stats", name=f"st{rb}")
            ssum_b = ssb.tile([1, RB], BF16, tag="ssumb", name=f"ssb{rb}")
            nc.scalar.copy(stats[:, 0, :], ssum[:])
            nc.scalar.copy(stats[:, 1, :], qsum[:])
            nc.scalar.copy(ssum_b[:], ssum[:])
            # -mu broadcast via tiny PE matmul (const -1/H stationary)
            negmu = opo.tile([128, RB], F32, tag="po", name=f"negmu{rb}")
            nc.tensor.matmul(negmu[:], negk[:], ssum_b[:],
                             start=True, stop=True)
            # lazy 1/std chain (consumed only after the out projection)
            t = ssb.tile([1, RB], F32, tag="t", name=f"t{rb}")
            t2 = ssb.tile([1, RB], F32, tag="t2", name=f"t2{rb}")
            u = ssb.tile([1, RB], F32, tag="u", name=f"u{rb}")
            std = ssb.tile([1, RB], F32, tag="t", name=f"std{rb}")
            rstd_f = ssb.tile([1, RB], F32, tag="t2", name=f"rstdf{rb}")
            rstd = ssb.tile([1, RB], BF16, tag="rstdb", name=f"rstd{rb}",
                            bufs=2)
            rs_s = esb.tile([128, RB], BF16, tag="rss", bufs=2,
                            name=f"rss{rb}")
            nc.vector.tensor_scalar_mul(t[:], stats[:, 0, :], 1.0 / HID)
            nc.vector.tensor_mul(t2[:], t[:], t[:])
            nc.vector.scalar_tensor_tensor(
                u[:], stats[:, 1, :], 1.0 / HID, t2[:],
                op0=mybir.AluOpType.mult, op1=mybir.AluOpType.subtract)
            nc.scalar.activation(std[:], u[:], AF.Sqrt, bias=eps_t[:])
            nc.vector.reciprocal_approx_fast(rstd_f[:], std[:])
            nc.vector.tensor_copy(rstd[:], rstd_f[:])
            nc.gpsimd.partition_broadcast(rs_s[:], rstd[:])
            # gate: (ao - mu) * silu(U);  1/std applied post-projection
            gated = esb.tile([128, 6, RB], BF16, tag="gated", bufs=1)
            for ct in range(6):
                d1 = esb.tile([128, RB], BF16, tag="d1")
                nc.vector.tensor_add(d1[:], aotf[:, ct, :], negmu[:])
                nc.vector.tensor_mul(gated[:, ct, :], d1[:],
                                     ut_sb[:, ct, r0:r1])
            rt_t = esb.tile([128, 3, RB], F32, tag="resid", bufs=1)
            nc.sync.dma_start(out=rt_t[:], in_=residT_r[:, :, r0:r1])
            o_t = esb.tile([128, 3, RB], F32, tag="osb", bufs=1)
            for oc in range(NPAIR):
                po = opo.tile([128, RB], F32, tag="po", name=f"po{oc}")
                for ct in range(6):
                    nc.tensor.matmul(po[:], wo_sb[:, ct, oc * 128:(oc + 1) * 128],
                                     gated[:, ct, :],
                                     start=(ct == 0), stop=(ct == 5))
                nc.vector.tensor_mul(o_t[:, oc, :], po[:], rs_s[:])
            nc.vector.tensor_add(o_t[:], o_t[:], rt_t[:])
            nc.gpsimd.dma_start(out=out_r[:, :, r0:r1], in_=o_t[:])

        # ---------------- schedule ------------------------------------
        # The AllGather for block i fires mid-attention as pairs finish;
        # epilogue(i) is emitted after attn(i+1) so the collective hides
        # under a full attention block.
        qt0 = proj_qk(0)
        proj_vu(0)
        ag0 = attn(0, qt0)
        qt1 = proj_qk(1)
        proj_vu(1)
        ag1 = attn(1, qt1)
        epilogue(0, ag0)
        qt2 = proj_qk(2)
        proj_vu(2)
        ag2 = attn(2, qt2)
        epilogue(1, ag1)
        qt3 = proj_qk(3)
        proj_vu(3)
        ag3 = attn(3, qt3)
        epilogue(2, ag2)
        epilogue(3, ag3)


# ---------------------------------------------------------------------------
# host side
# ---------------------------------------------------------------------------

def prep_inputs(x, attn_mask, W_proj, b_proj, ln_gamma, ln_beta, W_out, b_out):
    x = np.asarray(x, dtype=np.float32)
    W_proj = np.asarray(W_proj, dtype=np.float32)
    b_proj = np.asarray(b_proj, dtype=np.float32)
    ln_gamma = np.asarray(ln_gamma, dtype=np.float32)
    ln_beta = np.asarray(ln_beta, dtype=np.float32)
    W_out = np.asarray(W_out, dtype=np.float32)
    b_out = np.asarray(b_out, dtype=np.float32)

    tril = np.tril(np.ones((S, S), dtype=bool))
    am = np.asarray(attn_mask)
    if not all(np.array_equal(am[b], tril) for b in range(am.shape[0])):
        raise ValueError("kernel specialized for causal attn_mask")
    if np.any(b_proj != 0) or np.any(ln_beta != 0):
        raise ValueError("kernel specialized for zero b_proj / ln_beta")

    bf = ml_dtypes.bfloat16
    cos, sin = _rope_tables()
    sinN = sin.copy()
    sinN[:, 0:32] = -sinN[:, 0:32]
    cosT = np.tile(cos.T, (2, 1)).astype(bf)           # [128, S]
    sinT = np.tile(sinN.T, (2, 1)).astype(bf)

    pmat = np.zeros((128, 128), dtype=np.float32)      # rotate-half perm
    for h in range(2):
        b0 = 64 * h
        for d in range(64):
            pmat[b0 + d, b0 + (d + 32) % 64] = 1.0
    pmat = pmat.astype(bf)

    maskb = np.triu(np.ones((128, 128), dtype=np.float32)).astype(bf)
    ones_k = np.ones((128, 1), dtype=bf)

    Wg = (ln_gamma[:, None] * W_out).astype(np.float32)   # gamma folded
    U_c, V_c, Q_c, K_c = 0, HID, 2 * HID, 3 * HID

    in_maps = []
    for c in range(N_CORES):
        b, hh = c // 2, c % 2
        h0 = NH * hh * D                               # 384*hh col offset
        xTb = x[b].T                                   # [768, 2048]
        residT = (xTb[hh * 384:(hh + 1) * 384, :]
                  + b_out[hh * 384:(hh + 1) * 384, None]).astype(np.float32)
        in_maps.append(dict(
            xT=np.ascontiguousarray(xTb).astype(bf),
            wq=np.ascontiguousarray(W_proj[:, Q_c + h0:Q_c + h0 + 384]).astype(bf),
            wk=np.ascontiguousarray(W_proj[:, K_c + h0:K_c + h0 + 384]).astype(bf),
            wv=np.ascontiguousarray(W_proj[:, V_c + h0:V_c + h0 + 384]).astype(bf),
            wu=np.ascontiguousarray(W_proj[:, U_c:U_c + HID]).astype(bf),
            wo=np.ascontiguousarray(Wg[:, hh * 384:(hh + 1) * 384]).astype(bf),
            cosT=cosT, sinT=sinT, pmat=pmat, maskb=maskb, ones_k=ones_k,
            residT=np.ascontiguousarray(residT),
        ))
    return in_maps


def assemble(results, B=4):
    full = np.empty((B, S, HID), dtype=np.float32)
    for c in range(N_CORES):
        b, hh = c // 2, c % 2
        full[b, :, hh * 384:(hh + 1) * 384] = results[c]["out"].T
    return full


_NC_CACHE = {}


def get_nc(ndev=N_CORES):
    if ndev not in _NC_CACHE:
        pairs = [[i, i + 1] for i in range(0, ndev, 2)]
        _NC_CACHE[ndev] = build_nc(ndev, pairs)
    return _NC_CACHE[ndev]


def kernel(**inputs):
    in_maps = prep_inputs(**inputs)
    nc = get_nc(N_CORES)
    res = bass_utils.run_bass_kernel_spmd(
        nc, in_maps, core_ids=list(range(N_CORES)))
    return assemble(res.results)


# revision 22
# speedup vs baseline: 1.0586x; 1.0586x over previous
"""Sparse (sigmoid) attention block on 8 TRN2 NeuronCores.

Sharding: core c = (batch b=c//2, head-half hh=c%2).  Each core computes
QKV projection + RoPE + causal sigmoid-attention for its 6 heads over the
full 2048-row sequence of its batch; a per-row-block AllGather inside the
pair exchanges attention outputs so each core sees the full hidden dim,
then LayerNorm + silu(U) gating run locally and the output projection is
column-split across the pair (single collective per block, fired at the
earliest possible moment so it hides under the next attention block).

Layouts: Q^T/K^T are produced DIRECTLY in transposed [head-pair-dim, seq]
layout by making the projection weights PE-stationary; RoPE's rotate-half
becomes a host-built 128x128 permutation matmul (P @ Q^T) so no PE
transposes are needed anywhere.  Causal structure is exploited at 128-row
granularity inside each 512-query block: diagonal key-chunks trim their
scores / sigmoid / A@V work to the unmasked column range.  The LN mean is
broadcast via a tiny PE matmul with a constant -1/H stationary vector;
the 1/std scale is applied AFTER the output projection (it commutes with
the contraction) so the sqrt chain stays off the critical path.  All
heavy compute in bf16 with f32 PSUM accumulation.
"""

import numpy as np
import ml_dtypes

import concourse.bass as bass
import concourse.bacc as bacc
import concourse.mybir as mybir
import concourse.tile as tile
from concourse import bass_utils

BF16 = mybir.dt.bfloat16
F32 = mybir.dt.float32
AF = mybir.ActivationFunctionType

S = 2048          # sequence length
HID = 768         # hidden
D = 64            # head dim
NH = 6            # heads per core
NPAIR = 3         # head pairs per core
NRB = 4           # row blocks of 512
RB = 512
LN_EPS = 1e-8
N_CORES = 8


def _rope_tables():
    inv_freq = 1.0 / (10000.0 ** (np.arange(0, D, 2, dtype=np.float64) / D))
    t = np.arange(S, dtype=np.float64)
    freqs = np.outer(t, inv_freq)                      # [S, 32]
    emb = np.concatenate([freqs, freqs], axis=-1)      # [S, 64]
    return np.cos(emb).astype(np.float32), np.sin(emb).astype(np.float32)


def build_nc(ndev, pairs):
    """Emit the per-core Bass/Tile graph (identical for every core)."""
    nc = bacc.Bacc("TRN2", target_bir_lowering=False, debug=False,
                   num_devices=ndev)

    def din(name, shape, dt):
        return nc.dram_tensor(name, shape, dt, kind="ExternalInput").ap()

    xT = din("xT", [HID, S], BF16)
    wq = din("wq", [HID, 384], BF16)
    wk = din("wk", [HID, 384], BF16)
    wv = din("wv", [HID, 384], BF16)
    wu = din("wu", [HID, HID], BF16)                   # full U cols
    wo = din("wo", [HID, 384], BF16)                   # full rows, own cols
    cosT = din("cosT", [128, S], BF16)
    sinT = din("sinT", [128, S], BF16)                 # sign-folded sin^T
    pmat = din("pmat", [128, 128], BF16)               # rotate-half perm
    maskb = din("maskb", [128, 128], BF16)             # j>=i upper-tri
    ones_k = din("ones_k", [128, 1], BF16)
    residT = din("residT", [384, S], F32)              # x^T half + b_out
    out = nc.dram_tensor("out", [384, S], F32, kind="ExternalOutput").ap()

    xT_r = xT.rearrange("(k p) s -> p k s", p=128)     # [128, 6, S]
    wq_r = wq.rearrange("(k p) c -> p k c", p=128)     # [128, 6, 384]
    wk_r = wk.rearrange("(k p) c -> p k c", p=128)
    wv_r = wv.rearrange("(k p) c -> p k c", p=128)
    wu_r = wu.rearrange("(k p) c -> p k c", p=128)     # [128, 6, 768]
    wo_r = wo.rearrange("(k p) c -> p k c", p=128)     # [128, 6, 384]
    residT_r = residT.rearrange("(c p) s -> p c s", p=128)  # [128, 3, S]
    out_r = out.rearrange("(c p) s -> p c s", p=128)

    with tile.TileContext(nc) as tc:
        _emit(nc, tc, pairs, xT_r, wq_r, wk_r, wv_r, wu_r, wo_r,
              cosT, sinT, pmat, maskb, ones_k, residT_r, out_r)
    nc.compile()
    return nc


def _emit(nc, tc, pairs, xT_r, wq_r, wk_r, wv_r, wu_r, wo_r,
          cosT, sinT, pmat, maskb, ones_k, residT_r, out_r):
    from contextlib import ExitStack
    es = ExitStack()
    with es:
        # ---- resident SBUF tensors -----------------------------------
        res = es.enter_context(tc.tile_pool(name="resident", bufs=1))
        xT_sb = res.tile([128, 6, S], BF16, tag="xT")
        wq_sb = res.tile([128, 6, 384], BF16, tag="wq")
        wk_sb = res.tile([128, 6, 384], BF16, tag="wk")
        wv_sb = res.tile([128, 6, 384], BF16, tag="wv")
        wu_sb = res.tile([128, 6, HID], BF16, tag="wu")
        wo_sb = res.tile([128, 6, 384], BF16, tag="wo")
        cosT_sb = res.tile([128, S], BF16, tag="cosT")
        sinT_sb = res.tile([128, S], BF16, tag="sinT")
        pmat_sb = res.tile([128, 128], BF16, tag="pmat")
        maskb_sb = res.tile([128, 128], BF16, tag="maskb")
        ones_k_sb = res.tile([128, 1], BF16, tag="onesk")
        warm_sb = res.tile([128, 128], BF16, tag="warm")
        kt_sb = [res.tile([128, NPAIR, RB], BF16, tag=f"kt{i}", name=f"kt{i}")
                 for i in range(NRB)]                  # K^T slabs (roped)
        v_sb = [res.tile([128, 4, NH * D], BF16, tag=f"v{i}", name=f"v{i}")
                for i in range(NRB)]                   # V row-major slabs
        ut_sb = res.tile([128, 6, S], BF16, tag="ut")  # silu(U)^T full

        # critical-path loads first: wq + first x block start the PE
        nc.gpsimd.memset(warm_sb[:], 0.0)
        for k in range(6):
            nc.sync.dma_start(out=wq_sb[:, k, :], in_=wq_r[:, k, :])
        for k in range(6):
            nc.sync.dma_start(out=xT_sb[:, k, 0:RB], in_=xT_r[:, k, 0:RB])
        nc.sync.dma_start(out=pmat_sb[:], in_=pmat[:])
        nc.sync.dma_start(out=cosT_sb[:], in_=cosT[:])
        nc.sync.dma_start(out=sinT_sb[:], in_=sinT[:])
        for k in range(6):
            nc.sync.dma_start(out=wk_sb[:, k, :], in_=wk_r[:, k, :])
            nc.sync.dma_start(out=wv_sb[:, k, :], in_=wv_r[:, k, :])
            nc.sync.dma_start(out=wu_sb[:, k, :], in_=wu_r[:, k, :])
        nc.sync.dma_start(out=maskb_sb[:], in_=maskb[:])
        nc.sync.dma_start(out=ones_k_sb[:], in_=ones_k[:])
        for rb in range(1, NRB):
            for k in range(6):
                nc.sync.dma_start(out=xT_sb[:, k, rb * RB:(rb + 1) * RB],
                                  in_=xT_r[:, k, rb * RB:(rb + 1) * RB])
        for k in range(6):
            nc.sync.dma_start(out=wo_sb[:, k, :], in_=wo_r[:, k, :])

        # ---- pools ---------------------------------------------------
        dram = es.enter_context(tc.tile_pool(name="ccdram", bufs=4,
                                             space="DRAM"))
        projp = es.enter_context(tc.tile_pool(name="projp", bufs=2,
                                              space="PSUM"))      # 2 banks
        scp = es.enter_context(tc.tile_pool(name="scp", bufs=2,
                                            space="PSUM"))        # 4 banks
        avp = es.enter_context(tc.tile_pool(name="avp", bufs=1,
                                            space="PSUM"))        # 1 bank
        opo = es.enter_context(tc.tile_pool(name="opo", bufs=1,
                                            space="PSUM"))        # 1 bank
        psb = es.enter_context(tc.tile_pool(name="psb", bufs=2))
        atp = es.enter_context(tc.tile_pool(name="atp", bufs=8))
        qtp = es.enter_context(tc.tile_pool(name="qtp", bufs=2))
        aop = es.enter_context(tc.tile_pool(name="aop", bufs=2))
        esb = es.enter_context(tc.tile_pool(name="esb", bufs=2))
        ssb = es.enter_context(tc.tile_pool(name="ssb", bufs=1))

        eps_t = ssb.tile([1, 1], F32, tag="eps")
        nc.gpsimd.memset(eps_t[:], LN_EPS)
        negk = ssb.tile([1, 128], BF16, tag="negk")    # bcast lhsT: -1/HID
        nc.gpsimd.memset(negk[:], -1.0 / HID)

        # PE warm-up: ramp the clock while the first DMAs land
        wp = projp.tile([128, RB], F32, tag="pq", name="warm")
        for i in range(16):
            nc.tensor.matmul(wp[:, 0:128], warm_sb[:], warm_sb[:],
                             start=True, stop=True)

        # ---------------- per-phase emitters --------------------------
        # Emission is generator-based: attention pulls "background" PE
        # units (next block's projections, previous block's epilogue)
        # between its key-chunks so the PE instruction stream stays dense
        # while the Scalar engine grinds through the sigmoids.
        qts = {}

        def _finish_qk(st, r0, r1):
            qsb, dst, p = st
            pperm = projp.tile([128, RB], F32, tag="pq", name="pperm")
            nc.tensor.matmul(pperm[:], pmat_sb[:], qsb[:],
                             start=True, stop=True)
            t1 = psb.tile([128, RB], BF16, tag="t1")
            t2 = psb.tile([128, RB], BF16, tag="t2")
            nc.vector.tensor_mul(t1[:], qsb[:], cosT_sb[:, r0:r1])
            nc.vector.tensor_mul(t2[:], pperm[:], sinT_sb[:, r0:r1])
            nc.vector.tensor_add(dst[:, p, :], t1[:], t2[:])

        def proj_gen(rb):
            """Yields after each PE accumulation unit."""
            r0, r1 = rb * RB, (rb + 1) * RB
            qt = qtp.tile([128, NPAIR, RB], BF16, tag="qt", name=f"qt{rb}")
            qts[rb] = qt
            units = [(wq_sb, qt, p) for p in range(NPAIR)]
            units += [(wk_sb, kt_sb[rb], p) for p in range(NPAIR)]
            staged = []
            for w_sb, dst, p in units:
                pq = projp.tile([128, RB], F32, tag="pq", name=f"pq{p}")
                for k in range(6):
                    nc.tensor.matmul(pq[:], w_sb[:, k, p * 128:(p + 1) * 128],
                                     xT_sb[:, k, r0:r1],
                                     start=(k == 0), stop=(k == 5))
                qsb = psb.tile([128, RB], BF16, tag="qsb")
                nc.vector.tensor_copy(qsb[:], pq[:])
                staged.append((qsb, dst, p))
                if len(staged) == 2:
                    _finish_qk(staged.pop(0), r0, r1)
                yield
            while staged:
                _finish_qk(staged.pop(0), r0, r1)
            for rt4 in range(4):
                c0 = r0 + rt4 * 128
                pv = projp.tile([128, RB], F32, tag="pq", name="pv")
                for k in range(6):
                    nc.tensor.matmul(pv[:, 0:384], xT_sb[:, k, c0:c0 + 128],
                                     wv_sb[:, k, :],
                                     start=(k == 0), stop=(k == 5))
                nc.scalar.copy(v_sb[rb][:, rt4, :], pv[:, 0:384])
                yield
            for ct in range(6):
                pu = projp.tile([128, RB], F32, tag="pq", name="pu")
                for k in range(6):
                    nc.tensor.matmul(pu[:], wu_sb[:, k, ct * 128:(ct + 1) * 128],
                                     xT_sb[:, k, r0:r0 + RB],
                                     start=(k == 0), stop=(k == 5))
                usig = psb.tile([128, RB], BF16, tag="usig")
                nc.scalar.activation(usig[:], pu[:], AF.Sigmoid)
                nc.vector.tensor_mul(ut_sb[:, ct, r0:r0 + RB], usig[:], pu[:])
                yield

        def epi_gen(rb, ag_out):
            """Full-hidden LN + gate locally; out-proj own column half."""
            r0, r1 = rb * RB, (rb + 1) * RB
            aot = esb.tile([128, 2, NPAIR, RB], BF16, tag="aot")
            nc.sync.dma_start(out=aot[:],
                              in_=ag_out.rearrange("r p i j -> i r p j"))
            aotf = aot.rearrange("i r p j -> i (r p) j")   # [128, 6, RB]
            ssum = projp.tile([1, RB], F32, tag="pq", name=f"ssum{rb}")
            qsum = projp.tile([1, RB], F32, tag="pq", name=f"qsum{rb}")
            for ct in range(6):
                sq = psb.tile([128, RB], BF16, tag="sq")
                nc.vector.tensor_mul(sq[:], aotf[:, ct, :], aotf[:, ct, :])
                nc.tensor.matmul(ssum[:], ones_k_sb[:], aotf[:, ct, :],
                                 start=(ct == 0), stop=(ct == 5))
                nc.tensor.matmul(qsum[:], ones_k_sb[:], sq[:],
                                 start=(ct == 0), stop=(ct == 5))
            stats = ssb.tile([1, 2, RB], F32, tag="stats", name=f"st{rb}")
            ssum_b = ssb.tile([1, RB], BF16, tag="ssumb", name=f"ssb{rb}")
            nc.scalar.copy(stats[:, 0, :], ssum[:])
            nc.scalar.copy(stats[:, 1, :], qsum[:])
            nc.scalar.copy(ssum_b[:], ssum[:])
            yield
            # -mu broadcast via tiny PE matmul (const -1/H stationary)
            negmu = opo.tile([128, RB], F32, tag="po", name=f"negmu{rb}")
            nc.tensor.matmul(negmu[:], negk[:], ssum_b[:],
                             start=True, stop=True)
            # lazy 1/std chain (consumed only after the out projection)
            t = ssb.tile([1, RB], F32, tag="t", name=f"t{rb}")
            t2 = ssb.tile([1, RB], F32, tag="t2", name=f"t2{rb}")
            u = ssb.tile([1, RB], F32, tag="u", name=f"u{rb}")
            std = ssb.tile([1, RB], F32, tag="t", name=f"std{rb}")
            rstd_f = ssb.tile([1, RB], F32, tag="t2", name=f"rstdf{rb}")
            rstd = ssb.tile([1, RB], BF16, tag="rstdb", name=f"rstd{rb}",
                            bufs=2)
            rs_s = esb.tile([128, RB], BF16, tag="rss", bufs=2,
                            name=f"rss{rb}")
            nc.vector.tensor_scalar_mul(t[:], stats[:, 0, :], 1.0 / HID)
            nc.vector.tensor_mul(t2[:], t[:], t[:])
            nc.vector.scalar_tensor_tensor(
                u[:], stats[:, 1, :], 1.0 / HID, t2[:],
                op0=mybir.AluOpType.mult, op1=mybir.AluOpType.subtract)
            nc.scalar.activation(std[:], u[:], AF.Sqrt, bias=eps_t[:])
            nc.vector.reciprocal_approx_fast(rstd_f[:], std[:])
            nc.vector.tensor_copy(rstd[:], rstd_f[:])
            nc.gpsimd.partition_broadcast(rs_s[:], rstd[:])
            # gate: (ao - mu) * silu(U);  1/std applied post-projection
            gated = esb.tile([128, 6, RB], BF16, tag="gated", bufs=1)
            for ct in range(6):
                d1 = esb.tile([128, RB], BF16, tag="d1")
                nc.vector.tensor_add(d1[:], aotf[:, ct, :], negmu[:])
                nc.vector.tensor_mul(gated[:, ct, :], d1[:],
                                     ut_sb[:, ct, r0:r1])
            yield
            rt_t = esb.tile([128, 3, RB], F32, tag="resid", bufs=1)
            nc.sync.dma_start(out=rt_t[:], in_=residT_r[:, :, r0:r1])
            o_t = esb.tile([128, 3, RB], F32, tag="osb", bufs=1)
            for oc in range(NPAIR):
                po = opo.tile([128, RB], F32, tag="po", name=f"po{oc}")
                for ct in range(6):
                    nc.tensor.matmul(po[:], wo_sb[:, ct, oc * 128:(oc + 1) * 128],
                                     gated[:, ct, :],
                                     start=(ct == 0), stop=(ct == 5))
                nc.vector.tensor_mul(o_t[:, oc, :], po[:], rs_s[:])
                yield
            nc.vector.tensor_add(o_t[:], o_t[:], rt_t[:])
            nc.gpsimd.dma_start(out=out_r[:, :, r0:r1], in_=o_t[:])

        from collections import deque
        bgA = deque()        # projection units (dependency-free)
        bgB = deque()        # epilogue units (gated on the AllGather)

        def drive(allow_epi):
            while bgA:
                try:
                    next(bgA[0])
                    return True
                except StopIteration:
                    bgA.popleft()
            if allow_epi and bgB:
                try:
                    next(bgB[0])
                    return True
                except StopIteration:
                    bgB.popleft()
                    return drive(allow_epi)
            return False

        def drain():
            while drive(True):
                pass

        def attn(qb):
            """Causal sigmoid attention for query block qb, all pairs.
            Pulls background PE units between key-chunks."""
            qt = qts[qb]
            nkc = 4 * qb + 4
            it = [0]
            ag_in = dram.tile([NPAIR, 128, RB], BF16, tag="agin")
            ag_out = dram.tile([2, NPAIR, 128, RB], BF16, tag="agout")
            ao = aop.tile([128, NPAIR, RB], BF16, tag="ao", name=f"ao{qb}")
            for p in range(NPAIR):
                av = avp.tile([128, RB], F32, tag="av")
                ats = {}

                def _av(kc):
                    t = kc - 4 * qb
                    w0 = max(t, 0) * 128
                    at = ats.pop(kc)
                    for h01 in range(2):
                        b0 = 64 * h01
                        nc.tensor.matmul(
                            av[b0:b0 + 64, w0:RB],
                            v_sb[kc // 4][:, kc % 4,
                                          (2 * p + h01) * D:(2 * p + h01 + 1) * D],
                            at[:, h01, w0:RB],
                            start=(kc == 0), stop=(kc == nkc - 1),
                            skip_group_check=True)

                for kc in range(nkc):
                    t = kc - 4 * qb          # >=0: diagonal-region chunk
                    w0 = max(t, 0) * 128
                    sc = scp.tile([128, 2, RB], F32, tag="sc")
                    at = atp.tile([128, 2, RB], BF16, tag="at")
                    kslc = kt_sb[kc // 4]
                    c0 = (kc % 4) * 128
                    for h01 in range(2):
                        b0 = 64 * h01
                        nc.tensor.matmul(
                            sc[:, h01, w0:RB],
                            kslc[b0:b0 + 64, p, c0:c0 + 128],
                            qt[b0:b0 + 64, p, w0:RB],
                            start=True, stop=True)
                    nc.scalar.activation(at[:, :, w0:RB], sc[:, :, w0:RB],
                                         AF.Sigmoid, scale=0.125)
                    if t >= 0:
                        for h01 in range(2):
                            nc.vector.tensor_mul(at[:, h01, w0:w0 + 128],
                                                 at[:, h01, w0:w0 + 128],
                                                 maskb_sb[:])
                    ats[kc] = at
                    if kc >= 7:              # bound live `at` tiles
                        _av(kc - 7)
                    it[0] += 1
                    drive(allow_epi=it[0] >= 8)
                for kc in sorted(ats):
                    _av(kc)
                nc.vector.tensor_copy(ao[:, p, :], av[:])
                nc.gpsimd.dma_start(out=ag_in[p, :, :], in_=ao[:, p, :])
            nc.gpsimd.collective_compute(
                "AllGather", mybir.AluOpType.bypass, replica_groups=pairs,
                ins=[ag_in.opt()], outs=[ag_out.opt()])
            return ag_out

        # ---------------- schedule ------------------------------------
        for _ in proj_gen(0):
            pass
        ags = {}
        for qb in range(NRB):
            if qb + 1 < NRB:
                bgA.append(proj_gen(qb + 1))
            if qb - 1 >= 0:
                bgB.append(epi_gen(qb - 1, ags[qb - 1]))
            ags[qb] = attn(qb)
            while bgA:           # next block's projection must complete
                drive(False)
        drain()
        for _ in epi_gen(NRB - 1, ags[NRB - 1]):
            pass

# ---------------------------------------------------------------------------
# host side
# ---------------------------------------------------------------------------

def prep_inputs(x, attn_mask, W_proj, b_proj, ln_gamma, ln_beta, W_out, b_out):
    x = np.asarray(x, dtype=np.float32)
    W_proj = np.asarray(W_proj, dtype=np.float32)
    b_proj = np.asarray(b_proj, dtype=np.float32)
    ln_gamma = np.asarray(ln_gamma, dtype=np.float32)
    ln_beta = np.asarray(ln_beta, dtype=np.float32)
    W_out = np.asarray(W_out, dtype=np.float32)
    b_out = np.asarray(b_out, dtype=np.float32)

    tril = np.tril(np.ones((S, S), dtype=bool))
    am = np.asarray(attn_mask)
    if not all(np.array_equal(am[b], tril) for b in range(am.shape[0])):
        raise ValueError("kernel specialized for causal attn_mask")
    if np.any(b_proj != 0) or np.any(ln_beta != 0):
        raise ValueError("kernel specialized for zero b_proj / ln_beta")

    bf = ml_dtypes.bfloat16
    cos, sin = _rope_tables()
    sinN = sin.copy()
    sinN[:, 0:32] = -sinN[:, 0:32]
    cosT = np.tile(cos.T, (2, 1)).astype(bf)           # [128, S]
    sinT = np.tile(sinN.T, (2, 1)).astype(bf)

    pmat = np.zeros((128, 128), dtype=np.float32)      # rotate-half perm
    for h in range(2):
        b0 = 64 * h
        for d in range(64):
            pmat[b0 + d, b0 + (d + 32) % 64] = 1.0
    pmat = pmat.astype(bf)

    maskb = np.triu(np.ones((128, 128), dtype=np.float32)).astype(bf)
    ones_k = np.ones((128, 1), dtype=bf)

    Wg = (ln_gamma[:, None] * W_out).astype(np.float32)   # gamma folded
    U_c, V_c, Q_c, K_c = 0, HID, 2 * HID, 3 * HID

    in_maps = []
    for c in range(N_CORES):
        b, hh = c // 2, c % 2
        h0 = NH * hh * D                               # 384*hh col offset
        xTb = x[b].T                                   # [768, 2048]
        residT = (xTb[hh * 384:(hh + 1) * 384, :]
                  + b_out[hh * 384:(hh + 1) * 384, None]).astype(np.float32)
        in_maps.append(dict(
            xT=np.ascontiguousarray(xTb).astype(bf),
            wq=np.ascontiguousarray(W_proj[:, Q_c + h0:Q_c + h0 + 384]).astype(bf),
            wk=np.ascontiguousarray(W_proj[:, K_c + h0:K_c + h0 + 384]).astype(bf),
            wv=np.ascontiguousarray(W_proj[:, V_c + h0:V_c + h0 + 384]).astype(bf),
            wu=np.ascontiguousarray(W_proj[:, U_c:U_c + HID]).astype(bf),
            wo=np.ascontiguousarray(Wg[:, hh * 384:(hh + 1) * 384]).astype(bf),
            cosT=cosT, sinT=sinT, pmat=pmat, maskb=maskb, ones_k=ones_k,
            residT=np.ascontiguousarray(residT),
        ))
    return in_maps


def assemble(results, B=4):
    full = np.empty((B, S, HID), dtype=np.float32)
    for c in range(N_CORES):
        b, hh = c // 2, c % 2
        full[b, :, hh * 384:(hh + 1) * 384] = results[c]["out"].T
    return full


_NC_CACHE = {}


def get_nc(ndev=N_CORES):
    if ndev not in _NC_CACHE:
        pairs = [[i, i + 1] for i in range(0, ndev, 2)]
        _NC_CACHE[ndev] = build_nc(ndev, pairs)
    return _NC_CACHE[ndev]


def kernel(**inputs):
    in_maps = prep_inputs(**inputs)
    nc = get_nc(N_CORES)
    res = bass_utils.run_bass_kernel_spmd(
        nc, in_maps, core_ids=list(range(N_CORES)))
    return assemble(res.results)
